# revision 1
# baseline (speedup 1.0000x reference)
"""Trainium2 Bass kernel for the DecoderRNN-DTP problem.

Math (per reference):
  x   = relu(dtp_features @ W_enc.T + b_enc)              [B, H]
  gi_l = x @ W_ih_l.T + b_ih_l                            [B, 3H]   (constant over steps)
  60 steps, each step threads one hidden state h through 3 GRU cells:
      gh = h @ W_hh_l.T + b_hh_l
      r = sig(gi_r + gh_r); z = sig(gi_z + gh_z)
      n = tanh(gi_n + r * gh_n);  h = (1-z)*n + z*h
  y_t = h @ W_out.T + b_out                               [B, 4]
  out[b, o*60+t] = y_t[b, o]

Distribution: data-parallel over batch, 8 cores x 256 rows; weights replicated.

Layout is fully transposed on device: hidden/gate dims on SBUF partitions,
batch is the free/moving dim.  The 256-row per-core batch is further split
into two 128-row streams so the tensor engine can run one stream's W_hh
matmuls while the vector/scalar/gpsimd engines chew the other stream's gate
math -- the GRU recurrence is strictly serial within a stream, so two
independent streams are what buys engine overlap.

Precision: the recurrent W_hh matmuls use bf16 weights x bf16 h-copy
(numpy study: rel err ~5e-4 end-to-end because gate math and the h state
stay fp32); everything else (encoder, gi, preloads, y head, gate math) is
fp32/fp32r (fp22 multiply).  Gate constants (gi + biases) are preloaded
into PSUM by bank-wide identity matmuls so the W_hh accumulation lands on
them; exactly one start=True per PSUM bank (start clears has_written for
the whole bank).
"""

import numpy as np
import ml_dtypes

import concourse.bass as bass
import concourse.bacc as bacc
import concourse.tile as tile
import concourse.mybir as mybir
from concourse.bass_utils import run_bass_kernel_spmd

H = 512
B = 2048
STEPS = 60
NCORES = 8
BL = B // NCORES   # 256 batch rows per core
HB = BL // 2       # 128 rows per stream
KT = H // 128      # 4 contraction tiles over hidden dim
GT = 3 * H // 128  # 12 gate tiles
ET = 2048 // 128   # 16 contraction tiles over encoder input dim
YCHUNK = 6         # steps buffered between output DMAs

F32 = mybir.dt.float32
F32R = mybir.dt.float32r
BF16 = mybir.dt.bfloat16
AF = mybir.ActivationFunctionType
OP = mybir.AluOpType

_BUILT = None
LAST_RESULTS = None
LAST_IN_MAPS = None
REPS = 1  # timing builds repeat the step loop to cancel dispatch overhead


def r32(ap):
    return ap.bitcast(F32R)


def flat(ap):
    return ap.rearrange("p a b -> p (a b)")


def _build(reps=None):
    reps = REPS if reps is None else reps
    nc = bacc.Bacc("TRN2", target_bir_lowering=False, debug=False,
                   num_devices=NCORES)

    # ---- DRAM parameters (pre-laid-out on host) ----
    dtpT_d = nc.dram_tensor("dtpT", [ET, 128, BL], F32R, kind="ExternalInput")
    wencT_d = nc.dram_tensor("wencT", [ET, 128, H], F32R, kind="ExternalInput")
    wihT_d = nc.dram_tensor("wihT", [3, KT, 128, 3 * H], F32R, kind="ExternalInput")
    whhT_d = nc.dram_tensor("whhT", [3, KT, 128, 3 * H], BF16, kind="ExternalInput")
    woutT_d = nc.dram_tensor("woutT", [KT, 128, 4], F32R, kind="ExternalInput")
    # biases packed with layer l at partition 32*l (K=1 matmul base_partition
    # must be 0/32/64); bmi: p0 = b_enc, p32 = b_out
    brz_d = nc.dram_tensor("brz", [128, 2 * H], F32R, kind="ExternalInput")
    bni_d = nc.dram_tensor("bni", [128, H], F32R, kind="ExternalInput")
    bmi_d = nc.dram_tensor("bmi", [128, H], F32R, kind="ExternalInput")
    ident_d = nc.dram_tensor("ident", [128, 128], F32R, kind="ExternalInput")
    ones_d = nc.dram_tensor("ones", [128, BL], F32R, kind="ExternalInput")
    # b_hh n-chunk broadcast across a 128-wide half-batch, per layer
    bnhbc_d = nc.dram_tensor("bnhbc", [3, 128, 4, HB], F32R, kind="ExternalInput")
    h0_d = nc.dram_tensor("h0", [128, KT, BL], F32R, kind="ExternalInput")
    h0b_d = nc.dram_tensor("h0b", [128, KT, BL], BF16, kind="ExternalInput")
    y_d = nc.dram_tensor("y", [STEPS, 4, BL], F32, kind="ExternalOutput")

    with tile.TileContext(nc) as tc:
        with (
            tc.tile_pool(name="consts", bufs=1) as consts,
            tc.tile_pool(name="whh", bufs=1) as whhp,
            tc.tile_pool(name="wstream", bufs=3) as wstream,
            tc.tile_pool(name="wihstream", bufs=2) as wihstream,
            tc.tile_pool(name="gpool", bufs=3) as gpool,
            tc.tile_pool(name="hpool", bufs=2) as hpool,
            tc.tile_pool(name="work", bufs=1) as work,
            tc.tile_pool(name="ybuf", bufs=2) as ybufp,
            tc.tile_pool(name="pg", bufs=1, space=bass.MemorySpace.PSUM) as pg,
            tc.tile_pool(name="py", bufs=2, space=bass.MemorySpace.PSUM) as pyp,
        ):
            # ---- constants ----
            ident = consts.tile([128, 128], F32R)
            nc.sync.dma_start(ident[:, :], ident_d[:, :])
            ones = consts.tile([128, BL], F32R)
            nc.sync.dma_start(ones[:, :], ones_d[:, :])
            brz = consts.tile([128, 2 * H], F32R)
            nc.sync.dma_start(brz[:, :], brz_d[:, :])
            bni = consts.tile([128, H], F32R)
            nc.sync.dma_start(bni[:, :], bni_d[:, :])
            bmi = consts.tile([128, H], F32R)
            nc.sync.dma_start(bmi[:, :], bmi_d[:, :])
            bnhbc = consts.tile([128, 3, 4, HB], F32R)
            for l in range(3):
                nc.sync.dma_start(bnhbc[:, l, :, :], bnhbc_d[l, :, :, :])
            woutT = consts.tile([128, KT, 4], F32R)
            for kt in range(KT):
                nc.sync.dma_start(woutT[:, kt, :], woutT_d[kt, :, :])
            zeros2 = consts.tile([128, KT, HB], F32R)
            nc.sync.dma_start(zeros2[:, :, :], h0_d[:, :, 0:HB])

            # ---- recurrent weights, bf16 (resident) ----
            whhT = whhp.tile([128, 3, KT, 3 * H], BF16)
            for l in range(3):
                for kt in range(KT):
                    nc.sync.dma_start(whhT[:, l, kt, :], whhT_d[l, kt, :, :])

            # ---- encoder: xT = relu(W_enc @ dtp.T + b_enc), per stream ----
            psx = [pg.tile([128, KT, HB], F32, tag=f"g{s}", name=f"psx{s}")
                   for s in (0, 1)]
            for s in (0, 1):
                nc.tensor.matmul(flat(psx[s][:, :, :]), r32(ident[:, :]),
                                 r32(flat(zeros2[:, :, :])),
                                 start=True, stop=False)
            for et in range(ET):
                dtc = wstream.tile([128, BL], F32R, tag="dtp")
                nc.sync.dma_start(dtc[:, :], dtpT_d[et, :, :])
                wec = wstream.tile([128, H], F32R, tag="wenc")
                nc.sync.dma_start(wec[:, :], wencT_d[et, :, :])
                for s in (0, 1):
                    for ht in range(KT):
                        nc.tensor.matmul(psx[s][:, ht, :],
                                         r32(wec[:, ht * 128:(ht + 1) * 128]),
                                         r32(dtc[:, s * HB:(s + 1) * HB]),
                                         start=False, stop=False)
            for s in (0, 1):
                for ht in range(KT):
                    nc.tensor.matmul(psx[s][:, ht, :],
                                     r32(bmi[0:1, ht * 128:(ht + 1) * 128]),
                                     r32(ones[0:1, 0:HB]),
                                     start=False, stop=True)
            xT = work.tile([128, KT, BL], F32R, tag="xT")
            for s in (0, 1):
                nc.scalar.activation(xT[:, :, s * HB:(s + 1) * HB],
                                     psx[s][:, :, :], AF.Relu)

            # ---- G_l = gi_l (+ rz: +b_ih+b_hh ; n: +b_ih), per stream ----
            G = []
            for l in range(3):
                psg = [pg.tile([128, GT, HB], F32, tag=f"g{s}",
                                name=f"psg{l}_{s}") for s in (0, 1)]
                for s in (0, 1):
                    for bank in range(3):
                        nc.tensor.matmul(
                            flat(psg[s][:, 4 * bank:4 * bank + 4, :]),
                            r32(ident[:, :]), r32(flat(zeros2[:, :, :])),
                            start=True, stop=False)
                for kt in range(KT):
                    wic = wihstream.tile([128, 3 * H], F32R, tag="wih")
                    nc.sync.dma_start(wic[:, :], wihT_d[l, kt, :, :])
                    for s in (0, 1):
                        for gt in range(GT):
                            nc.tensor.matmul(
                                psg[s][:, gt, :],
                                r32(wic[:, gt * 128:(gt + 1) * 128]),
                                r32(xT[:, kt, s * HB:(s + 1) * HB]),
                                start=False, stop=False)
                for s in (0, 1):
                    for gt in range(8):
                        nc.tensor.matmul(psg[s][:, gt, :],
                                         r32(brz[32 * l:32 * l + 1,
                                                 gt * 128:(gt + 1) * 128]),
                                         r32(ones[32 * l:32 * l + 1, 0:HB]),
                                         start=False, stop=True)
                    for j in range(4):
                        nc.tensor.matmul(psg[s][:, 8 + j, :],
                                         r32(bni[32 * l:32 * l + 1,
                                                 j * 128:(j + 1) * 128]),
                                         r32(ones[32 * l:32 * l + 1, 0:HB]),
                                         start=False, stop=True)
                g = gpool.tile([128, 2, GT, HB], F32R, tag="G")
                for s in (0, 1):
                    nc.scalar.copy(g[:, s, :, :], psg[s][:, :, :])
                G.append(g)

            # ---- recurrent loop: 60 steps x 3 GRU cells, 2 streams ----
            h32 = hpool.tile([128, KT, BL], F32R, tag="h32")
            nc.sync.dma_start(h32[:, :, :], h0_d[:, :, :])
            hb = hpool.tile([128, KT, BL], BF16, tag="hb")
            nc.sync.dma_start(hb[:, :, :], h0b_d[:, :, :])

            ybuf = None
            for t in range(STEPS * reps):
                t = t % STEPS
                for l in range(3):
                    h32n = hpool.tile([128, KT, BL], F32R, tag="h32")
                    hbn = hpool.tile([128, KT, BL], BF16, tag="hb")
                    for s in (0, 1):
                        c0, c1 = s * HB, (s + 1) * HB
                        ps = pg.tile([128, GT, HB], F32, tag=f"g{s}")
                        # bank-wide gate-constant preloads (one start=True per
                        # 2KB PSUM bank -- start clears the whole bank's
                        # has_written bits)
                        for bank in range(2):
                            nc.tensor.matmul(
                                flat(ps[:, 4 * bank:4 * bank + 4, :]),
                                r32(ident[:, :]),
                                r32(flat(G[l][:, s, 4 * bank:4 * bank + 4, :])),
                                start=True, stop=False)
                        nc.tensor.matmul(
                            flat(ps[:, 8:12, :]), r32(ident[:, :]),
                            r32(flat(bnhbc[:, l, :, :])),
                            start=True, stop=False)
                        # W_hh @ h accumulation, bf16
                        for gt in range(GT):
                            for kt in range(KT):
                                nc.tensor.matmul(
                                    ps[:, gt, :],
                                    whhT[:, l, kt, gt * 128:(gt + 1) * 128],
                                    hb[:, kt, c0:c1],
                                    start=False, stop=(kt == KT - 1))
                        # gate math (fp32)
                        rz = work.tile([128, 8, HB], F32, tag=f"rz{s}")
                        nc.scalar.activation(rz[:, :, :], ps[:, 0:8, :],
                                             AF.Sigmoid)
                        tt = work.tile([128, 4, HB], F32, tag=f"t{s}")
                        nc.vector.tensor_tensor(tt[:, :, :], ps[:, 8:12, :],
                                                rz[:, 0:4, :], OP.mult)
                        uu = work.tile([128, 4, HB], F32, tag=f"u{s}")
                        nc.gpsimd.tensor_tensor(uu[:, :, :], tt[:, :, :],
                                                G[l][:, s, 8:12, :], OP.add)
                        nn_ = work.tile([128, 4, HB], F32, tag=f"n{s}")
                        nc.scalar.activation(nn_[:, :, :], uu[:, :, :], AF.Tanh)
                        aa = work.tile([128, 4, HB], F32, tag=f"a{s}")
                        nc.vector.tensor_tensor(aa[:, :, :], h32[:, :, c0:c1],
                                                nn_[:, :, :], OP.subtract)
                        bb = work.tile([128, 4, HB], F32, tag=f"b{s}")
                        nc.vector.tensor_tensor(bb[:, :, :], aa[:, :, :],
                                                rz[:, 4:8, :], OP.mult)
                        nc.vector.tensor_tensor(h32n[:, :, c0:c1], nn_[:, :, :],
                                                bb[:, :, :], OP.add)
                        nc.gpsimd.tensor_copy(hbn[:, :, c0:c1],
                                              h32n[:, :, c0:c1])
                    h32, hb = h32n, hbn

                # y_t = W_out @ h + b_out   -> [4, 256]
                py = pyp.tile([4, BL], F32, tag="y")
                for kt in range(KT):
                    nc.tensor.matmul(py[:, :], r32(woutT[:, kt, :]),
                                     r32(h32[:, kt, :]),
                                     start=(kt == 0), stop=False)
                nc.tensor.matmul(py[:, :], r32(bmi[32:33, 0:4]),
                                 r32(ones[32:33, :]), start=False, stop=True)
                if t % YCHUNK == 0:
                    ybuf = ybufp.tile([4, YCHUNK, BL], F32, tag="yb")
                nc.scalar.copy(ybuf[0:4, t % YCHUNK, :], py[:, :])
                if (t + 1) % YCHUNK == 0:
                    c0 = t + 1 - YCHUNK
                    nc.sync.dma_start(
                        y_d[c0:c0 + YCHUNK, :, :].rearrange("t o b -> o t b"),
                        ybuf[0:4, :, :])

    nc.compile()
    return nc


def _get_built():
    global _BUILT
    if _BUILT is None:
        _BUILT = _build()
    return _BUILT


def _pack(rows, n):
    out = np.zeros((128, n), np.float32)
    for i, r in enumerate(rows):
        out[32 * i, :] = np.asarray(r, np.float32)
    return out


def kernel(**inputs):
    global LAST_RESULTS, LAST_IN_MAPS
    nc = _get_built()

    f = np.float32
    bf = ml_dtypes.bfloat16
    dtp = np.ascontiguousarray(inputs["dtp_features"], dtype=f)
    shared = {
        "wencT": np.ascontiguousarray(
            inputs["W_enc"].T.reshape(ET, 128, H).astype(f)),
        "wihT": np.ascontiguousarray(np.stack(
            [inputs[f"W_ih{l}"].T.reshape(KT, 128, 3 * H) for l in "123"]).astype(f)),
        "whhT": np.ascontiguousarray(np.stack(
            [inputs[f"W_hh{l}"].T.reshape(KT, 128, 3 * H) for l in "123"]).astype(bf)),
        "woutT": np.ascontiguousarray(
            inputs["W_out"].T.reshape(KT, 128, 4).astype(f)),
        "brz": _pack([(inputs[f"b_ih{l}"] + inputs[f"b_hh{l}"])[:2 * H]
                      for l in "123"], 2 * H),
        "bni": _pack([inputs[f"b_ih{l}"][2 * H:] for l in "123"], H),
        "bmi": _pack([inputs["b_enc"], np.pad(inputs["b_out"], (0, H - 4))], H),
        "ident": np.eye(128, dtype=f),
        "ones": np.ones((128, BL), f),
        "bnhbc": np.ascontiguousarray(np.broadcast_to(
            np.stack([inputs[f"b_hh{l}"][2 * H:].reshape(4, 128).T
                      for l in "123"])[:, :, :, None],
            (3, 128, 4, HB)).astype(f)),
        "h0": np.zeros((128, KT, BL), f),
        "h0b": np.zeros((128, KT, BL), bf),
    }
    in_maps = []
    for c in range(NCORES):
        m = dict(shared)
        m["dtpT"] = np.ascontiguousarray(
            dtp[c * BL:(c + 1) * BL].T.reshape(ET, 128, BL))
        in_maps.append(m)

    LAST_IN_MAPS = in_maps
    res = run_bass_kernel_spmd(nc, in_maps, core_ids=list(range(NCORES)))
    LAST_RESULTS = res
    outs = []
    for c in range(NCORES):
        y = res.results[c]["y"]  # [60, 4, 256]
        outs.append(np.transpose(y, (2, 1, 0)).reshape(BL, 4 * STEPS))
    return np.ascontiguousarray(np.concatenate(outs, axis=0), dtype=np.float32)



# revision 8
# speedup vs baseline: 2.2598x; 2.2598x over previous
"""Trainium2 Bass kernel for the DecoderRNN-DTP problem.

Math (per reference):
  x   = relu(dtp_features @ W_enc.T + b_enc)              [B, H]
  gi_l = x @ W_ih_l.T + b_ih_l                            [B, 3H]   (constant over steps)
  60 steps, each step threads one hidden state h through 3 GRU cells:
      gh = h @ W_hh_l.T + b_hh_l
      r = sig(gi_r + gh_r); z = sig(gi_z + gh_z)
      n = tanh(gi_n + r * gh_n);  h = (1-z)*n + z*h
  y_t = h @ W_out.T + b_out                               [B, 4]
  out[b, o*60+t] = y_t[b, o]

Distribution: data-parallel over batch, 8 cores x 256 rows; weights replicated.

Layout is fully transposed on device: hidden/gate dims on SBUF partitions,
batch is the free/moving dim.  The 256-row per-core batch is split into two
128-row streams so the tensor engine runs one stream's W_hh matmuls while the
vector engines chew the other stream's gate math.

Per (cell, stream) the critical chain after the W_hh matmuls is only
  tt = ps_n * r  ->  uu = tt + gi_n  ->  n = tanh(uu)
  ->  v = (z-1)*n (fused scalar_tensor_tensor)  ->  hb = p - v  (bf16, DVE)
with p = z*h_old and sig(r)/sig(z) computed during the matmuls (r tiles are
accumulated first), and the fp32 h' = p - v kept off the critical path on
the Pool engine (only needed by the NEXT cell's p and the y head).

Prologue matmuls (encoder, gi) run 256 wide: fp32r streams 1 row/cycle only
when the moving free dim is >= 256 (4x penalty below that).

Precision: recurrent W_hh matmuls and the tiny y head use bf16 weights x
bf16 h-copy; gate math and the carried h state stay fp32.  Gate constants
(gi + biases) are preloaded into PSUM by bank-wide identity matmuls so the
W_hh accumulation lands on them; exactly one start=True per PSUM bank
(start clears has_written for the whole bank).
"""

import numpy as np
import ml_dtypes

import concourse.bass as bass
import concourse.bacc as bacc
import concourse.tile as tile
import concourse.mybir as mybir
from concourse.bass_utils import run_bass_kernel_spmd

H = 512
B = 2048
STEPS = 60
NCORES = 8
BL = B // NCORES   # 256 batch rows per core
HB = BL // 2       # 128 rows per stream
KT = H // 128      # 4 contraction tiles over hidden dim
GT = 3 * H // 128  # 12 gate tiles
ET = 2048 // 128   # 16 contraction tiles over encoder input dim
YCHUNK = 6         # steps buffered between output DMAs

F32 = mybir.dt.float32
F32R = mybir.dt.float32r
BF16 = mybir.dt.bfloat16
AF = mybir.ActivationFunctionType
OP = mybir.AluOpType

_BUILT = None
LAST_RESULTS = None
LAST_IN_MAPS = None
REPS = 1  # timing builds repeat the step loop to cancel dispatch overhead


def r32(ap):
    return ap.bitcast(F32R)


def flat(ap):
    return ap.rearrange("p a b -> p (a b)")


def _build(reps=None):
    reps = REPS if reps is None else reps
    nc = bacc.Bacc("TRN2", target_bir_lowering=False, debug=False,
                   num_devices=NCORES)

    # ---- DRAM parameters (pre-laid-out on host) ----
    dtpT_d = nc.dram_tensor("dtpT", [ET, 128, BL], F32R, kind="ExternalInput")
    wencT_d = nc.dram_tensor("wencT", [ET, 128, H], F32R, kind="ExternalInput")
    wihT_d = nc.dram_tensor("wihT", [3, KT, 128, 3 * H], F32R, kind="ExternalInput")
    whhT_d = nc.dram_tensor("whhT", [3, KT, 128, 3 * H], BF16, kind="ExternalInput")
    woutTb_d = nc.dram_tensor("woutTb", [KT, 128, 4], BF16, kind="ExternalInput")
    # biases packed with layer l at partition 32*l (K=1 matmul base_partition
    # must be 0/32/64); bmi: p0 = b_enc, p32 = b_out
    brz_d = nc.dram_tensor("brz", [128, 2 * H], F32R, kind="ExternalInput")
    bni_d = nc.dram_tensor("bni", [128, H], F32R, kind="ExternalInput")
    bmi_d = nc.dram_tensor("bmi", [128, H], F32R, kind="ExternalInput")
    bmib_d = nc.dram_tensor("bmib", [128, H], BF16, kind="ExternalInput")
    ident_d = nc.dram_tensor("ident", [128, 128], F32R, kind="ExternalInput")
    ones_d = nc.dram_tensor("ones", [128, BL], F32R, kind="ExternalInput")
    onesb_d = nc.dram_tensor("onesb", [128, BL], BF16, kind="ExternalInput")
    # b_hh n-chunk broadcast across a 128-wide half-batch, per layer
    bnhbc_d = nc.dram_tensor("bnhbc", [3, 128, 4, HB], F32R, kind="ExternalInput")
    h0_d = nc.dram_tensor("h0", [128, KT, BL], F32R, kind="ExternalInput")
    h0b_d = nc.dram_tensor("h0b", [128, KT, BL], BF16, kind="ExternalInput")
    y_d = nc.dram_tensor("y", [STEPS, 4, BL], F32, kind="ExternalOutput")

    with tile.TileContext(nc) as tc:
        with (
            tc.tile_pool(name="consts", bufs=1) as consts,
            tc.tile_pool(name="whh", bufs=1) as whhp,
            tc.tile_pool(name="wstream", bufs=3) as wstream,
            tc.tile_pool(name="wihstream", bufs=2) as wihstream,
            tc.tile_pool(name="gpool", bufs=1) as gpool,
            tc.tile_pool(name="hpool", bufs=2) as hpool,
            tc.tile_pool(name="work", bufs=1) as work,
            tc.tile_pool(name="ybuf", bufs=2) as ybufp,
            tc.tile_pool(name="pg", bufs=1, space=bass.MemorySpace.PSUM) as pg,
            tc.tile_pool(name="py", bufs=2, space=bass.MemorySpace.PSUM) as pyp,
        ):
            # ---- constants ----
            ident = consts.tile([128, 128], F32R)
            nc.sync.dma_start(ident[:, :], ident_d[:, :])
            ones = consts.tile([128, BL], F32R)
            nc.sync.dma_start(ones[:, :], ones_d[:, :])
            onesb = consts.tile([128, BL], BF16)
            nc.sync.dma_start(onesb[:, :], onesb_d[:, :])
            brz = consts.tile([128, 2 * H], F32R)
            nc.sync.dma_start(brz[:, :], brz_d[:, :])
            bni = consts.tile([128, H], F32R)
            nc.sync.dma_start(bni[:, :], bni_d[:, :])
            bmi = consts.tile([128, H], F32R)
            nc.sync.dma_start(bmi[:, :], bmi_d[:, :])
            bmib = consts.tile([128, H], BF16)
            nc.sync.dma_start(bmib[:, :], bmib_d[:, :])
            bnhbc = consts.tile([128, 3, 4, HB], F32R)
            for l in range(3):
                nc.sync.dma_start(bnhbc[:, l, :, :], bnhbc_d[l, :, :, :])
            woutTb = consts.tile([128, KT, 4], BF16)
            for kt in range(KT):
                nc.sync.dma_start(woutTb[:, kt, :], woutTb_d[kt, :, :])
            zeros2 = consts.tile([128, KT, BL], F32R)
            nc.sync.dma_start(zeros2[:, :, :], h0_d[:, :, :])

            # ---- recurrent weights, bf16 (resident) ----
            whhT = whhp.tile([128, 3, KT, 3 * H], BF16)
            for l in range(3):
                for kt in range(KT):
                    nc.sync.dma_start(whhT[:, l, kt, :], whhT_d[l, kt, :, :])

            # ---- encoder: xT = relu(W_enc @ dtp.T + b_enc), 256 wide ----
            psx = pg.tile([128, KT, BL], F32, tag="g0", name="psx")
            for bank in range(2):
                nc.tensor.matmul(flat(psx[:, 2 * bank:2 * bank + 2, :]),
                                 r32(ident[:, :]),
                                 r32(flat(zeros2[:, 0:2, :])),
                                 start=True, stop=False)
            for et in range(ET):
                dtc = wstream.tile([128, BL], F32R, tag="dtp")
                nc.sync.dma_start(dtc[:, :], dtpT_d[et, :, :])
                wec = wstream.tile([128, H], F32R, tag="wenc")
                nc.sync.dma_start(wec[:, :], wencT_d[et, :, :])
                for ht in range(KT):
                    nc.tensor.matmul(psx[:, ht, :],
                                     r32(wec[:, ht * 128:(ht + 1) * 128]),
                                     r32(dtc[:, :]),
                                     start=False, stop=False)
            for ht in range(KT):
                nc.tensor.matmul(psx[:, ht, :],
                                 r32(bmi[0:1, ht * 128:(ht + 1) * 128]),
                                 r32(ones[0:1, :]),
                                 start=False, stop=True)
            xT = work.tile([128, KT, BL], F32R, tag="xT")
            nc.scalar.activation(xT[:, :, :], psx[:, :, :], AF.Relu)

            # ---- G_l = gi_l (+ rz: +b_ih+b_hh ; n: +b_ih), 256 wide ----
            # computed in two 6-tile PSUM chunks; stored per stream for the
            # loop's bank-wide preloads
            G = []
            for l in range(3):
                wihl = wihstream.tile([128, KT, 3 * H], F32R, tag="wih")
                for kt in range(KT):
                    nc.sync.dma_start(wihl[:, kt, :], wihT_d[l, kt, :, :])
                g = gpool.tile([128, 2, GT, HB], F32R, tag=f"G{l}")
                for c in range(2):
                    psg = pg.tile([128, 6, BL], F32, tag=f"g{c}",
                                  name=f"psg{l}_{c}")
                    for bank in range(3):
                        nc.tensor.matmul(
                            flat(psg[:, 2 * bank:2 * bank + 2, :]),
                            r32(ident[:, :]), r32(flat(zeros2[:, 0:2, :])),
                            start=True, stop=False)
                    for kt in range(KT):
                        for j in range(6):
                            gt = 6 * c + j
                            nc.tensor.matmul(
                                psg[:, j, :],
                                r32(wihl[:, kt, gt * 128:(gt + 1) * 128]),
                                r32(xT[:, kt, :]),
                                start=False, stop=False)
                    for j in range(6):
                        gt = 6 * c + j
                        if gt < 8:
                            bsrc = brz[32 * l:32 * l + 1,
                                       gt * 128:(gt + 1) * 128]
                        else:
                            bsrc = bni[32 * l:32 * l + 1,
                                       (gt - 8) * 128:(gt - 7) * 128]
                        nc.tensor.matmul(psg[:, j, :], r32(bsrc),
                                         r32(ones[32 * l:32 * l + 1, :]),
                                         start=False, stop=True)
                    for s in (0, 1):
                        nc.scalar.copy(g[:, s, 6 * c:6 * c + 6, :],
                                       psg[:, :, s * HB:(s + 1) * HB])
                G.append(g)

            # ---- recurrent loop: 60 steps x 3 GRU cells, 2 streams ----
            h32 = hpool.tile([128, KT, BL], F32R, tag="h32")
            nc.sync.dma_start(h32[:, :, :], h0_d[:, :, :])
            hb = hpool.tile([128, KT, BL], BF16, tag="hb")
            nc.sync.dma_start(hb[:, :, :], h0b_d[:, :, :])

            ybuf = None
            for t in range(STEPS * reps):
                t = t % STEPS
                for l in range(3):
                    h32n = hpool.tile([128, KT, BL], F32R, tag="h32")
                    hbn = hpool.tile([128, KT, BL], BF16, tag="hb")
                    ps, rr, zz, pp, tt, uu, nn_, vv = [], [], [], [], [], [], [], []
                    for s in (0, 1):
                        c0, c1 = s * HB, (s + 1) * HB
                        p_ = pg.tile([128, GT, HB], F32, tag=f"g{s}")
                        ps.append(p_)
                        # bank-wide gate-constant preloads (one start=True per
                        # 2KB PSUM bank); banks: r = tiles 0-3, z = 4-7, n = 8-11
                        nc.tensor.matmul(
                            flat(p_[:, 0:4, :]), r32(ident[:, :]),
                            r32(flat(G[l][:, s, 0:4, :])),
                            start=True, stop=False)
                        nc.tensor.matmul(
                            flat(p_[:, 4:8, :]), r32(ident[:, :]),
                            r32(flat(G[l][:, s, 4:8, :])),
                            start=True, stop=False)
                        nc.tensor.matmul(
                            flat(p_[:, 8:12, :]), r32(ident[:, :]),
                            r32(flat(bnhbc[:, l, :, :])),
                            start=True, stop=False)
                        # W_hh @ h accumulation, bf16; r tiles first so sig(r)
                        # overlaps the z/n matmuls
                        for gt in range(GT):
                            for kt in range(KT):
                                nc.tensor.matmul(
                                    p_[:, gt, :],
                                    whhT[:, l, kt, gt * 128:(gt + 1) * 128],
                                    hb[:, kt, c0:c1],
                                    start=False, stop=(kt == KT - 1))
                    # gate math; emission order is per-engine queue order, so
                    # ops are laid out by data-ready time:
                    #   ACT : sigr0 sigz0 sigr1 tanh0 sigz1 tanh1
                    #   DVE : tt0 uu0 v0 hb0 tt1 uu1 v1 hb1
                    #   Pool: p0 p1 h32n0 h32n1
                    for s in (0, 1):
                        for lst, nm in ((rr, "r"), (zz, "z"), (tt, "t"),
                                        (pp, "p"), (uu, "u"), (nn_, "n"),
                                        (vv, "v")):
                            lst.append(work.tile([128, 4, HB], F32,
                                                 tag=f"{nm}{s}",
                                                 name=f"{nm}{s}"))

                    def chain_front(s):
                        c0, c1 = s * HB, (s + 1) * HB
                        nc.scalar.activation(rr[s][:, :, :], ps[s][:, 0:4, :],
                                             AF.Sigmoid)
                        nc.scalar.activation(zz[s][:, :, :], ps[s][:, 4:8, :],
                                             AF.Sigmoid)
                        nc.gpsimd.tensor_tensor(pp[s][:, :, :], zz[s][:, :, :],
                                                h32[:, :, c0:c1], OP.mult)
                        nc.vector.tensor_tensor(tt[s][:, :, :],
                                                ps[s][:, 8:12, :],
                                                rr[s][:, :, :], OP.mult)
                        nc.vector.tensor_tensor(uu[s][:, :, :], tt[s][:, :, :],
                                                G[l][:, s, 8:12, :], OP.add)

                    def chain_back(s):
                        c0, c1 = s * HB, (s + 1) * HB
                        nc.scalar.activation(nn_[s][:, :, :], uu[s][:, :, :],
                                             AF.Tanh)
                        # v = (z - 1) * n   (so h' = p - v = z*h + (1-z)*n)
                        nc.vector.scalar_tensor_tensor(
                            vv[s][:, :, :], zz[s][:, :, :], 1.0,
                            nn_[s][:, :, :], OP.subtract, OP.mult)
                        # bf16 h for the next cell's matmuls: critical path
                        nc.vector.tensor_tensor(hbn[:, :, c0:c1],
                                                pp[s][:, :, :], vv[s][:, :, :],
                                                OP.subtract)
                        # fp32 h off the critical path (next cell's p, y head)
                        nc.gpsimd.tensor_tensor(h32n[:, :, c0:c1],
                                                pp[s][:, :, :], vv[s][:, :, :],
                                                OP.subtract)

                    chain_front(0)
                    chain_back(0)
                    chain_front(1)
                    chain_back(1)
                    h32, hb = h32n, hbn

                # y_t = W_out @ h + b_out   -> [4, 256]  (bf16 weights x hb)
                py = pyp.tile([4, BL], F32, tag="y")
                for kt in range(KT):
                    nc.tensor.matmul(py[:, :], woutTb[:, kt, :],
                                     hb[:, kt, :],
                                     start=(kt == 0), stop=False)
                nc.tensor.matmul(py[:, :], bmib[32:33, 0:4],
                                 onesb[32:33, :], start=False, stop=True)
                if t % YCHUNK == 0:
                    ybuf = ybufp.tile([4, YCHUNK, BL], F32, tag="yb")
                nc.scalar.copy(ybuf[0:4, t % YCHUNK, :], py[:, :])
                if (t + 1) % YCHUNK == 0:
                    c0 = t + 1 - YCHUNK
                    nc.sync.dma_start(
                        y_d[c0:c0 + YCHUNK, :, :].rearrange("t o b -> o t b"),
                        ybuf[0:4, :, :])

    nc.compile()
    return nc


def _get_built():
    global _BUILT
    if _BUILT is None:
        _BUILT = _build()
    return _BUILT


def _pack(rows, n):
    out = np.zeros((128, n), np.float32)
    for i, r in enumerate(rows):
        out[32 * i, :] = np.asarray(r, np.float32)
    return out


def kernel(**inputs):
    global LAST_RESULTS, LAST_IN_MAPS
    nc = _get_built()

    f = np.float32
    bf = ml_dtypes.bfloat16
    dtp = np.ascontiguousarray(inputs["dtp_features"], dtype=f)
    bmi = _pack([inputs["b_enc"], np.pad(inputs["b_out"], (0, H - 4))], H)
    shared = {
        "wencT": np.ascontiguousarray(
            inputs["W_enc"].T.reshape(ET, 128, H).astype(f)),
        "wihT": np.ascontiguousarray(np.stack(
            [inputs[f"W_ih{l}"].T.reshape(KT, 128, 3 * H) for l in "123"]).astype(f)),
        "whhT": np.ascontiguousarray(np.stack(
            [inputs[f"W_hh{l}"].T.reshape(KT, 128, 3 * H) for l in "123"]).astype(bf)),
        "woutTb": np.ascontiguousarray(
            inputs["W_out"].T.reshape(KT, 128, 4).astype(bf)),
        "brz": _pack([(inputs[f"b_ih{l}"] + inputs[f"b_hh{l}"])[:2 * H]
                      for l in "123"], 2 * H),
        "bni": _pack([inputs[f"b_ih{l}"][2 * H:] for l in "123"], H),
        "bmi": bmi,
        "bmib": bmi.astype(bf),
        "ident": np.eye(128, dtype=f),
        "ones": np.ones((128, BL), f),
        "onesb": np.ones((128, BL), bf),
        "bnhbc": np.ascontiguousarray(np.broadcast_to(
            np.stack([inputs[f"b_hh{l}"][2 * H:].reshape(4, 128).T
                      for l in "123"])[:, :, :, None],
            (3, 128, 4, HB)).astype(f)),
        "h0": np.zeros((128, KT, BL), f),
        "h0b": np.zeros((128, KT, BL), bf),
    }
    in_maps = []
    for c in range(NCORES):
        m = dict(shared)
        m["dtpT"] = np.ascontiguousarray(
            dtp[c * BL:(c + 1) * BL].T.reshape(ET, 128, BL))
        in_maps.append(m)

    LAST_IN_MAPS = in_maps
    res = run_bass_kernel_spmd(nc, in_maps, core_ids=list(range(NCORES)))
    LAST_RESULTS = res
    outs = []
    for c in range(NCORES):
        y = res.results[c]["y"]  # [60, 4, 256]
        outs.append(np.transpose(y, (2, 1, 0)).reshape(BL, 4 * STEPS))
    return np.ascontiguousarray(np.concatenate(outs, axis=0), dtype=np.float32)
